# revision 27
# baseline (speedup 1.0000x reference)
"""Causal self-attention with RoPE on 8 Trainium2 NeuronCores.

Problem (hardcoded): B=2, S=2048, E=2048, H=16 heads, D=128 head dim.
  qkv = x @ W_qkv.T ; RoPE(q, k) ; causal softmax attention ; out @ W_out.T

Sharding: tensor-parallel over heads. Each of the 8 cores handles 2 heads
for both batches: computes its heads' q/k/v projections (column-sharded
W_qkv), runs attention, and produces a partial output projection
(row-sharded W_out). The host sums the 8 partial outputs.

Device-side layout choices:
 - Everything streams through the TensorEngine in bf16 (f32 PSUM accum).
 - x is passed pre-transposed as xT [E, T]; the qkv projection computes
   qkvT = Wc @ x.T directly in "feature-major" [feat, token] layout, which
   is exactly the rhs layout attention and the output projection need.
 - Attention computes transposed score tiles scoresT [k, q] so softmax'd
   probabilities feed the PV matmul with no transposes. Softmax denominators
   come from an ones-column matmul (column sums), and exp() skips
   max-subtraction (scores are O(1) for this problem: |s| < ~4).
 - Causality: only k-tiles with k <= q are computed; diagonal-band tiles
   get a multiplicative {0,1} mask after exp.
 - v needs [token, D] (lhsT) layout for PV; it is produced feature-major
   like q/k and flipped with TensorE 128x128 transposes.
 - The per-(batch, q-block) output projection is software-pipelined one
   unit behind attention so the softmax normalization chain
   (reciprocal -> partition_broadcast -> multiply) never stalls the PE.
"""

import math
from contextlib import ExitStack

import numpy as np
import ml_dtypes

import concourse.bass as bass
import concourse.mybir as mybir
import concourse.tile as tile
from concourse import bacc
from concourse.bass_utils import run_bass_kernel_spmd
from concourse.masks import make_identity

BF16 = mybir.dt.bfloat16
F32 = mybir.dt.float32
P = 128

# problem config
B, S, E = 2, 2048, 2048
H, D = 16, 128
N_CORES = 8
HPC = H // N_CORES  # heads per core = 2


def build_nc(b=B, s=S, e=E, hpc=HPC):
    """Build the per-core Bass program (same program on every core)."""
    T = b * s            # total tokens
    NT = T // 512        # 512-token blocks
    KE = e // P          # contraction tiles for the qkv projection
    MQKV = 3 * hpc       # qkv feature tiles per core (q,q,k,k,v,v for hpc=2)
    QT = s // 512        # 512-wide q blocks per batch
    KT = s // P          # 128-wide k blocks per batch
    ME = e // P          # output-embedding tiles

    nc = bacc.Bacc("TRN2", target_bir_lowering=False, debug=False)

    xT = nc.dram_tensor("xT", [P, KE, T], BF16, kind="ExternalInput").ap()
    wqkv = nc.dram_tensor("wqkv", [P, KE, MQKV * P], BF16, kind="ExternalInput").ap()
    wo = nc.dram_tensor("wo", [P, hpc, e], BF16, kind="ExternalInput").ap()
    cosq = nc.dram_tensor("cosq", [P, s], BF16, kind="ExternalInput").ap()
    sinq = nc.dram_tensor("sinq", [P, s], BF16, kind="ExternalInput").ap()
    cosk = nc.dram_tensor("cosk", [P, s], BF16, kind="ExternalInput").ap()
    sink = nc.dram_tensor("sink", [P, s], BF16, kind="ExternalInput").ap()
    bandmask = nc.dram_tensor("bandmask", [P, 896], BF16, kind="ExternalInput").ap()
    outT = nc.dram_tensor("outT", [e, T], F32, kind="ExternalOutput").ap()

    with tile.TileContext(nc) as tc, ExitStack() as ctx:
        persist = ctx.enter_context(tc.tile_pool(name="persist", bufs=1))
        attn_pool = ctx.enter_context(tc.tile_pool(name="attnstore", bufs=1))
        # phase-2 working pools allocated BEFORE the phase-1 pools so their
        # SBUF addresses don't overlap phase-1's (no release-zone stall at
        # the phase boundary).
        exp_pool = ctx.enter_context(tc.tile_pool(name="expp", bufs=4))
        small = ctx.enter_context(tc.tile_pool(name="small", bufs=3))
        qk_pool = tc.alloc_tile_pool(name="qkvstore", bufs=1)

        ident = persist.tile([P, P], BF16)
        make_identity(nc, ident)
        ones_col = persist.tile([P, 1], BF16)
        nc.vector.memset(ones_col, 1.0)
        mask_sb = persist.tile([P, 896], BF16)
        wo_sb = persist.tile([P, hpc, e], BF16)

        attn_sb = [attn_pool.tile([P, T], BF16, name=f"attnsb{h}") for h in range(hpc)]
        qk_sb = [qk_pool.tile([P, T], BF16, name=f"qksb{i}") for i in range(2 * hpc)]
        vblk = [qk_pool.tile([P, T // P, P], BF16, name=f"vblk{h}") for h in range(hpc)]

        # ---- phase 1: qkv projection + RoPE + v transpose ----
        with ExitStack() as p1:
            wpool = p1.enter_context(tc.tile_pool(name="wq", bufs=1))
            xpool = p1.enter_context(tc.tile_pool(name="xs", bufs=3))
            trig_pool = p1.enter_context(tc.tile_pool(name="trig", bufs=1))
            rope_pool = p1.enter_context(tc.tile_pool(name="rope", bufs=3))
            qkv_ps = p1.enter_context(tc.tile_pool(name="qkvps", bufs=4, space="PSUM"))
            tr_ps = p1.enter_context(tc.tile_pool(name="trps", bufs=2, space="PSUM"))

            # k-chunked weight/x DMAs so the first matmuls start early
            w_sb = wpool.tile([P, KE, MQKV * P], BF16)
            x_tiles = [None] * NT
            x_tiles[0] = xpool.tile([P, KE, 512], BF16, name="x_sb")
            quarter = MQKV * P // 4
            for q4 in range(4):
                nc.sync.dma_start(w_sb[:, 0, q4 * quarter:(q4 + 1) * quarter],
                                  wqkv[:, 0, q4 * quarter:(q4 + 1) * quarter])
                nc.sync.dma_start(x_tiles[0][:, 0, q4 * 128:(q4 + 1) * 128],
                                  xT[:, 0, q4 * 128:(q4 + 1) * 128])
            for k in range(1, KE):
                nc.sync.dma_start(w_sb[:, k, :], wqkv[:, k, :])
                nc.sync.dma_start(x_tiles[0][:, k, :], xT[:, k, 0:512])
            trig = {}
            for nm, ap in [("cosq", cosq), ("sinq", sinq), ("cosk", cosk), ("sink", sink)]:
                t = trig_pool.tile([P, s], BF16, name=nm + "_sb")
                nc.sync.dma_start(t, ap)
                trig[nm] = t
            nc.sync.dma_start(mask_sb, bandmask)
            nc.sync.dma_start(wo_sb, wo)

            for n in range(NT):
                x_sb = x_tiles[n]
                if x_sb is None:
                    x_sb = xpool.tile([P, KE, 512], BF16, name="x_sb")
                    nc.sync.dma_start(x_sb, xT[:, :, n * 512:(n + 1) * 512])
                s0 = (n % QT) * 512  # position offset within the batch
                for m in range(MQKV):
                    ps = qkv_ps.tile([P, 512], F32, name="qkvps")
                    for k in range(KE):
                        nc.tensor.matmul(
                            ps, w_sb[:, k, m * P:(m + 1) * P], x_sb[:, k, :],
                            start=(k == 0), stop=(k == KE - 1),
                        )
                    kind, h = m // hpc, m % hpc
                    if kind < 2:  # q or k: RoPE
                        raw = rope_pool.tile([P, 512], BF16, name="raw")
                        nc.scalar.copy(out=raw, in_=ps)
                        shuf = rope_pool.tile([P, 512], BF16, name="shuf")
                        nc.vector.tensor_copy(out=shuf[0:64], in_=raw[64:128])
                        nc.vector.tensor_copy(out=shuf[64:128], in_=raw[0:64])
                        c_t = trig["cosq" if kind == 0 else "cosk"][:, s0:s0 + 512]
                        s_t = trig["sinq" if kind == 0 else "sink"][:, s0:s0 + 512]
                        t1 = rope_pool.tile([P, 512], BF16, name="t1")
                        nc.vector.tensor_mul(t1, raw, c_t)
                        nc.vector.tensor_mul(shuf, shuf, s_t)
                        dst = qk_sb[kind * hpc + h][:, n * 512:(n + 1) * 512]
                        nc.vector.tensor_add(dst, t1, shuf)
                    else:  # v: cast then transpose into [token, D] blocks
                        vT = rope_pool.tile([P, 512], BF16, name="vT")
                        nc.scalar.copy(out=vT, in_=ps)
                        for t4 in range(4):
                            tp = tr_ps.tile([P, P], BF16, name="trp")
                            nc.tensor.transpose(tp, vT[:, t4 * P:(t4 + 1) * P], ident)
                            nc.vector.tensor_copy(out=vblk[h][:, n * 4 + t4, :], in_=tp)

        # ---- phase 2+3: attention with pipelined output projection ----
        with ExitStack() as p2:
            opool = p2.enter_context(tc.tile_pool(name="outp", bufs=3))
            dram_pool = p2.enter_context(tc.tile_pool(name="drbounce", bufs=3, space="DRAM"))
            p2b = p2.enter_context(ExitStack())
            sc_ps = p2b.enter_context(tc.tile_pool(name="scps", bufs=2, space="PSUM"))
            att_ps = p2b.enter_context(tc.tile_pool(name="attps", bufs=2, space="PSUM"))
            sum_ps = p2b.enter_context(tc.tile_pool(name="sumps", bufs=2, space="PSUM"))
            out_ps = p2b.enter_context(tc.tile_pool(name="outps", bufs=2, space="PSUM"))

            def emit_outproj(nt, alternate=False, pool=None):
                for mt in range(ME):
                    ps = (pool or out_ps).tile([P, 512], F32, name="ops")
                    for h in range(hpc):
                        nc.tensor.matmul(
                            ps, wo_sb[:, h, mt * P:(mt + 1) * P],
                            attn_sb[h][:, nt * 512:(nt + 1) * 512],
                            start=(h == 0), stop=(h == hpc - 1),
                        )
                    osb = opool.tile([P, 512], F32, name="osb")
                    if alternate and mt % 3 == 2:
                        nc.scalar.copy(out=osb, in_=ps)
                    else:
                        nc.vector.tensor_copy(out=osb, in_=ps)
                    nc.sync.dma_start(
                        outT[mt * P:(mt + 1) * P, nt * 512:(nt + 1) * 512], osb)

            units = [(bb, qt) for bb in range(b) for qt in range(QT)]
            prev_nt = None
            for bb, qt in units:
                nk = 4 * (qt + 1)  # causal: k-tiles 0 .. 4(qt+1)-1
                att_t, rb_t = [], []
                # both heads' kt loops interleaved: two independent matmul
                # chains keep the in-order PE busy while exp() runs
                atts = [att_ps.tile([P, 512], F32, name="att") for _ in range(hpc)]
                sms = [sum_ps.tile([1, 512], F32, name="sm") for _ in range(hpc)]
                qsls = [qk_sb[h][:, bb * s + qt * 512: bb * s + (qt + 1) * 512]
                        for h in range(hpc)]
                pair_es = [[] for _ in range(hpc)]
                sum_rhs = [[] for _ in range(hpc)]
                for kt in range(nk):
                    # causal raggedness: diagonal-band tile j only needs
                    # q-columns >= 128*j
                    j = kt - 4 * qt
                    off = max(0, 128 * j)
                    w_q = 512 - off
                    for h in range(hpc):
                        k_store = qk_sb[hpc + h]
                        sp = sc_ps.tile([P, 512], F32, name="sp")
                        nc.tensor.matmul(
                            sp[:, :w_q],
                            k_store[:, bb * s + kt * P: bb * s + (kt + 1) * P],
                            qsls[h][:, off:512], start=True, stop=True,
                        )
                        e_t = exp_pool.tile([P, 512], BF16, name="e_t", bufs=8)
                        nc.scalar.activation(
                            e_t[:, :w_q], sp[:, :w_q],
                            mybir.ActivationFunctionType.Exp)
                        if j >= 0:  # triangle block only
                            nc.vector.tensor_mul(
                                e_t[:, 0:128], e_t[:, 0:128], mask_sb[:, 384:512])
                        nc.tensor.matmul(
                            atts[h][:, off:512], vblk[h][:, bb * KT + kt, :],
                            e_t[:, :w_q],
                            start=(kt == 0), stop=(kt == nk - 1),
                        )
                        # denominators: non-diagonal tiles pair-summed on
                        # GpSimd; diagonal (ragged) tiles summed individually.
                        # The ones-matmuls run after the kt loop so the
                        # in-order PE stream never waits on the adds.
                        if j < 0:
                            pair_es[h].append(e_t)
                            if len(pair_es[h]) == 2:
                                tp = exp_pool.tile([P, 512], BF16, name="tp", bufs=10)
                                nc.gpsimd.tensor_add(tp, pair_es[h][0], pair_es[h][1])
                                sum_rhs[h].append((tp, 0))
                                pair_es[h] = []
                        else:
                            sum_rhs[h].append((e_t, off))
                for h in range(hpc):
                    sm = sms[h]
                    for qd, (tq, off) in enumerate(sum_rhs[h]):
                        nc.tensor.matmul(
                            sm[:, off:512], ones_col, tq[:, :512 - off],
                            start=(qd == 0), stop=(qd == len(sum_rhs[h]) - 1),
                        )
                    # reciprocal straight from PSUM (frees the bank), then
                    # broadcast across partitions via a DRAM bounce (DMA can
                    # replicate a DRAM row with a zero-stride partition dim)
                    r = small.tile([1, 512], F32, name="r")
                    nc.vector.reciprocal_approx_fast(out=r, in_=sm)
                    rd = dram_pool.tile([1, 512], F32, name="rd")
                    nc.sync.dma_start(rd, r)
                    rb = small.tile([P, 512], F32, name="rb")
                    rd_b = bass.AP(tensor=rd.tensor, offset=rd.offset,
                                   ap=[[0, P]] + list(rd.ap[1:]))
                    nc.sync.dma_start(rb, rd_b)
                    att_t.append(atts[h])
                    rb_t.append(rb)
                for h in range(hpc):
                    nc.vector.tensor_tensor(
                        attn_sb[h][:, bb * s + qt * 512: bb * s + (qt + 1) * 512],
                        att_t[h], rb_t[h], mybir.AluOpType.mult,
                    )
                if prev_nt is not None:
                    emit_outproj(prev_nt)
                prev_nt = bb * QT + qt
            emit_outproj(prev_nt, alternate=True)

        qk_pool.release()

    nc.compile()
    return nc


def make_common_inputs(x, b=B, s=S, e=E):
    """Inputs identical on every core: xT, trig tables, causal band mask."""
    T = b * s
    KE = e // P
    xflat = np.ascontiguousarray(x.reshape(T, e).T)        # [E, T] f32
    xT = np.ascontiguousarray(
        xflat.reshape(KE, P, T).transpose(1, 0, 2)).astype(ml_dtypes.bfloat16)

    inv_freq = (1.0 / (10000.0 ** (np.arange(0, D, 2, dtype=np.float32) / D)))
    t = np.arange(s, dtype=np.float32)
    freqs = np.outer(t, inv_freq)                           # [S, 64]
    cos = np.cos(freqs).astype(np.float32)                  # [S, 64]
    sin = np.sin(freqs).astype(np.float32)
    cosT = np.concatenate([cos, cos], axis=1).T             # [128, S]
    sinT = np.concatenate([sin, sin], axis=1).T
    sgn = np.where(np.arange(D) < D // 2, -1.0, 1.0).astype(np.float32)[:, None]
    scale = 1.0 / math.sqrt(D)
    cosq = np.ascontiguousarray(cosT * scale).astype(ml_dtypes.bfloat16)
    sinq = np.ascontiguousarray(sinT * sgn * scale).astype(ml_dtypes.bfloat16)
    cosk = np.ascontiguousarray(cosT).astype(ml_dtypes.bfloat16)
    sink = np.ascontiguousarray(sinT * sgn).astype(ml_dtypes.bfloat16)

    r = np.arange(P)[:, None]
    cc = np.arange(896)[None, :]
    bandmask = (cc >= r + 384).astype(ml_dtypes.bfloat16)

    return {
        "xT": xT, "cosq": cosq, "sinq": sinq, "cosk": cosk, "sink": sink,
        "bandmask": bandmask,
    }


def make_core_inputs(W_qkv, W_out, core, b=B, s=S, e=E, hpc=HPC):
    """Per-core column-sharded W_qkv (as lhsT tiles) and row-sharded W_out."""
    KE = e // P
    heads = [core * hpc + i for i in range(hpc)]
    rows = []
    for base in (0, e, 2 * e):  # q, k, v row blocks of W_qkv
        for h in heads:
            rows.append(W_qkv[base + h * D: base + (h + 1) * D])
    Wc = np.concatenate(rows, axis=0)                       # [3*hpc*128, E]
    WcT = np.ascontiguousarray(Wc.T)                        # [E, 3*hpc*128]
    wqkv = np.ascontiguousarray(
        WcT.reshape(KE, P, 3 * hpc * P).transpose(1, 0, 2)).astype(ml_dtypes.bfloat16)

    wo = np.stack(
        [np.ascontiguousarray(W_out[:, h * D:(h + 1) * D].T) for h in heads],
        axis=1)                                             # [128, hpc, E]
    wo = np.ascontiguousarray(wo).astype(ml_dtypes.bfloat16)
    return {"wqkv": wqkv, "wo": wo}


_NC_CACHE = {}


def get_nc():
    key = (B, S, E, HPC)
    if key not in _NC_CACHE:
        _NC_CACHE[key] = build_nc()
    return _NC_CACHE[key]


def kernel(x, W_qkv, W_out):
    x = np.asarray(x, dtype=np.float32)
    W_qkv = np.asarray(W_qkv, dtype=np.float32)
    W_out = np.asarray(W_out, dtype=np.float32)

    nc = get_nc()
    common = make_common_inputs(x)
    in_maps = [dict(common, **make_core_inputs(W_qkv, W_out, c))
               for c in range(N_CORES)]
    res = run_bass_kernel_spmd(nc, in_maps, list(range(N_CORES)))
    total = res.results[0]["outT"].astype(np.float32)
    for c in range(1, N_CORES):
        total = total + res.results[c]["outT"]
    return np.ascontiguousarray(total.T).reshape(B, S, E).astype(np.float32)


# revision 28
# speedup vs baseline: 1.0464x; 1.0464x over previous
"""Causal self-attention with RoPE on 8 Trainium2 NeuronCores.

Problem (hardcoded): B=2, S=2048, E=2048, H=16 heads, D=128 head dim.
  qkv = x @ W_qkv.T ; RoPE(q, k) ; causal softmax attention ; out @ W_out.T

Sharding: tensor-parallel over heads. Each of the 8 cores handles 2 heads
for both batches: computes its heads' q/k/v projections (column-sharded
W_qkv), runs attention, and produces a partial output projection
(row-sharded W_out). The host sums the 8 partial outputs.

Device-side layout choices:
 - Everything streams through the TensorEngine in bf16 (f32 PSUM accum).
 - x is passed pre-transposed as xT [E, T]; the qkv projection computes
   qkvT = Wc @ x.T directly in "feature-major" [feat, token] layout, which
   is exactly the rhs layout attention and the output projection need.
 - Attention computes transposed score tiles scoresT [k, q] so softmax'd
   probabilities feed the PV matmul with no transposes. Softmax denominators
   come from an ones-column matmul (column sums), and exp() skips
   max-subtraction (scores are O(1) for this problem: |s| < ~4).
 - Causality: only k-tiles with k <= q are computed; diagonal-band tiles
   get a multiplicative {0,1} mask after exp.
 - v needs [token, D] (lhsT) layout for PV; it is produced feature-major
   like q/k and flipped with TensorE 128x128 transposes.
 - The per-(batch, q-block) output projection is software-pipelined one
   unit behind attention so the softmax normalization chain
   (reciprocal -> partition_broadcast -> multiply) never stalls the PE.
"""

import math
from contextlib import ExitStack

import numpy as np
import ml_dtypes

import concourse.bass as bass
import concourse.mybir as mybir
import concourse.tile as tile
from concourse import bacc
from concourse.bass_utils import run_bass_kernel_spmd
from concourse.masks import make_identity

BF16 = mybir.dt.bfloat16
F32 = mybir.dt.float32
P = 128

# problem config
B, S, E = 2, 2048, 2048
H, D = 16, 128
N_CORES = 8
HPC = H // N_CORES  # heads per core = 2


def build_nc(b=B, s=S, e=E, hpc=HPC):
    """Build the per-core Bass program (same program on every core)."""
    T = b * s            # total tokens
    NT = T // 512        # 512-token blocks
    KE = e // P          # contraction tiles for the qkv projection
    MQKV = 3 * hpc       # qkv feature tiles per core (q,q,k,k,v,v for hpc=2)
    QT = s // 512        # 512-wide q blocks per batch
    KT = s // P          # 128-wide k blocks per batch
    ME = e // P          # output-embedding tiles

    nc = bacc.Bacc("TRN2", target_bir_lowering=False, debug=False)

    xT = nc.dram_tensor("xT", [P, KE, T], BF16, kind="ExternalInput").ap()
    wqkv = nc.dram_tensor("wqkv", [P, KE, MQKV * P], BF16, kind="ExternalInput").ap()
    wo = nc.dram_tensor("wo", [P, hpc, e], BF16, kind="ExternalInput").ap()
    cosq = nc.dram_tensor("cosq", [P, s], BF16, kind="ExternalInput").ap()
    sinq = nc.dram_tensor("sinq", [P, s], BF16, kind="ExternalInput").ap()
    cosk = nc.dram_tensor("cosk", [P, s], BF16, kind="ExternalInput").ap()
    sink = nc.dram_tensor("sink", [P, s], BF16, kind="ExternalInput").ap()
    bandmask = nc.dram_tensor("bandmask", [P, 896], BF16, kind="ExternalInput").ap()
    outT = nc.dram_tensor("outT", [e, T], F32, kind="ExternalOutput").ap()

    with tile.TileContext(nc) as tc, ExitStack() as ctx:
        persist = ctx.enter_context(tc.tile_pool(name="persist", bufs=1))
        attn_pool = ctx.enter_context(tc.tile_pool(name="attnstore", bufs=1))
        # phase-2 working pools allocated BEFORE the phase-1 pools so their
        # SBUF addresses don't overlap phase-1's (no release-zone stall at
        # the phase boundary).
        exp_pool = ctx.enter_context(tc.tile_pool(name="expp", bufs=4))
        small = ctx.enter_context(tc.tile_pool(name="small", bufs=3))
        qk_pool = tc.alloc_tile_pool(name="qkvstore", bufs=1)

        ident = persist.tile([P, P], BF16)
        make_identity(nc, ident)
        ones_col = persist.tile([P, 1], BF16)
        nc.vector.memset(ones_col, 1.0)
        mask_sb = persist.tile([P, 896], BF16)
        wo_sb = persist.tile([P, hpc, e], BF16)

        attn_sb = [attn_pool.tile([P, T], BF16, name=f"attnsb{h}") for h in range(hpc)]
        qk_sb = [qk_pool.tile([P, T], BF16, name=f"qksb{i}") for i in range(2 * hpc)]
        vblk = [qk_pool.tile([P, T // P, P], BF16, name=f"vblk{h}") for h in range(hpc)]

        # ---- phase 1: qkv projection + RoPE + v transpose ----
        with ExitStack() as p1:
            wpool = p1.enter_context(tc.tile_pool(name="wq", bufs=1))
            xpool = p1.enter_context(tc.tile_pool(name="xs", bufs=3))
            trig_pool = p1.enter_context(tc.tile_pool(name="trig", bufs=1))
            rope_pool = p1.enter_context(tc.tile_pool(name="rope", bufs=3))
            qkv_ps = p1.enter_context(tc.tile_pool(name="qkvps", bufs=4, space="PSUM"))
            tr_ps = p1.enter_context(tc.tile_pool(name="trps", bufs=2, space="PSUM"))

            # k-chunked weight/x DMAs so the first matmuls start early
            w_sb = wpool.tile([P, KE, MQKV * P], BF16)
            x_tiles = [None] * NT
            x_tiles[0] = xpool.tile([P, KE, 512], BF16, name="x_sb")
            quarter = MQKV * P // 4
            for q4 in range(4):
                nc.sync.dma_start(w_sb[:, 0, q4 * quarter:(q4 + 1) * quarter],
                                  wqkv[:, 0, q4 * quarter:(q4 + 1) * quarter])
                nc.sync.dma_start(x_tiles[0][:, 0, q4 * 128:(q4 + 1) * 128],
                                  xT[:, 0, q4 * 128:(q4 + 1) * 128])
            for k in range(1, KE):
                nc.sync.dma_start(w_sb[:, k, :], wqkv[:, k, :])
                nc.sync.dma_start(x_tiles[0][:, k, :], xT[:, k, 0:512])
            trig = {}
            for nm, ap in [("cosq", cosq), ("sinq", sinq), ("cosk", cosk), ("sink", sink)]:
                t = trig_pool.tile([P, s], BF16, name=nm + "_sb")
                nc.sync.dma_start(t, ap)
                trig[nm] = t
            nc.sync.dma_start(mask_sb, bandmask)
            nc.sync.dma_start(wo_sb, wo)

            for n in range(NT):
                x_sb = x_tiles[n]
                if x_sb is None:
                    x_sb = xpool.tile([P, KE, 512], BF16, name="x_sb")
                    nc.sync.dma_start(x_sb, xT[:, :, n * 512:(n + 1) * 512])
                s0 = (n % QT) * 512  # position offset within the batch
                for m in range(MQKV):
                    ps = qkv_ps.tile([P, 512], F32, name="qkvps")
                    for k in range(KE):
                        nc.tensor.matmul(
                            ps, w_sb[:, k, m * P:(m + 1) * P], x_sb[:, k, :],
                            start=(k == 0), stop=(k == KE - 1),
                        )
                    kind, h = m // hpc, m % hpc
                    if kind < 2:  # q or k: RoPE
                        raw = rope_pool.tile([P, 512], BF16, name="raw")
                        nc.scalar.copy(out=raw, in_=ps)
                        shuf = rope_pool.tile([P, 512], BF16, name="shuf")
                        nc.vector.tensor_copy(out=shuf[0:64], in_=raw[64:128])
                        nc.vector.tensor_copy(out=shuf[64:128], in_=raw[0:64])
                        c_t = trig["cosq" if kind == 0 else "cosk"][:, s0:s0 + 512]
                        s_t = trig["sinq" if kind == 0 else "sink"][:, s0:s0 + 512]
                        t1 = rope_pool.tile([P, 512], BF16, name="t1")
                        nc.vector.tensor_mul(t1, raw, c_t)
                        nc.vector.tensor_mul(shuf, shuf, s_t)
                        dst = qk_sb[kind * hpc + h][:, n * 512:(n + 1) * 512]
                        nc.vector.tensor_add(dst, t1, shuf)
                    else:  # v: cast then transpose into [token, D] blocks
                        vT = rope_pool.tile([P, 512], BF16, name="vT")
                        nc.scalar.copy(out=vT, in_=ps)
                        for t4 in range(4):
                            tp = tr_ps.tile([P, P], BF16, name="trp")
                            nc.tensor.transpose(tp, vT[:, t4 * P:(t4 + 1) * P], ident)
                            nc.vector.tensor_copy(out=vblk[h][:, n * 4 + t4, :], in_=tp)

        # ---- phase 2+3: attention with pipelined output projection ----
        with ExitStack() as p2:
            opool = p2.enter_context(tc.tile_pool(name="outp", bufs=3))
            dram_pool = p2.enter_context(tc.tile_pool(name="drbounce", bufs=3, space="DRAM"))
            p2b = p2.enter_context(ExitStack())
            sc_ps = p2b.enter_context(tc.tile_pool(name="scps", bufs=2, space="PSUM"))
            att_ps = p2b.enter_context(tc.tile_pool(name="attps", bufs=2, space="PSUM"))
            sum_ps = p2b.enter_context(tc.tile_pool(name="sumps", bufs=2, space="PSUM"))
            out_ps = p2b.enter_context(tc.tile_pool(name="outps", bufs=2, space="PSUM"))

            def emit_outproj(nt, alternate=False, pool=None):
                for mt in range(ME):
                    ps = (pool or out_ps).tile([P, 512], F32, name="ops")
                    for h in range(hpc):
                        nc.tensor.matmul(
                            ps, wo_sb[:, h, mt * P:(mt + 1) * P],
                            attn_sb[h][:, nt * 512:(nt + 1) * 512],
                            start=(h == 0), stop=(h == hpc - 1),
                        )
                    osb = opool.tile([P, 512], F32, name="osb")
                    if alternate and mt % 3 == 2:
                        nc.scalar.copy(out=osb, in_=ps)
                    else:
                        nc.vector.tensor_copy(out=osb, in_=ps)
                    nc.sync.dma_start(
                        outT[mt * P:(mt + 1) * P, nt * 512:(nt + 1) * 512], osb)

            units = [(bb, qt) for bb in range(b) for qt in range(QT)]
            prev_nt = None
            for bb, qt in units:
                nk = 4 * (qt + 1)  # causal: k-tiles 0 .. 4(qt+1)-1
                att_t, rb_t = [], []
                for h in range(hpc):
                    q_store = qk_sb[h]
                    k_store = qk_sb[hpc + h]
                    att = att_ps.tile([P, 512], F32, name="att")
                    sm = sum_ps.tile([1, 512], F32, name="sm")
                    qsl = q_store[:, bb * s + qt * 512: bb * s + (qt + 1) * 512]
                    pair_es, sum_rhs = [], []
                    for kt in range(nk):
                        # causal raggedness: diagonal-band tile j only needs
                        # q-columns >= 128*j
                        j = kt - 4 * qt
                        off = max(0, 128 * j)
                        w_q = 512 - off
                        sp = sc_ps.tile([P, 512], F32, name="sp")
                        nc.tensor.matmul(
                            sp[:, :w_q],
                            k_store[:, bb * s + kt * P: bb * s + (kt + 1) * P],
                            qsl[:, off:512], start=True, stop=True,
                        )
                        e_t = exp_pool.tile([P, 512], BF16, name="e_t", bufs=8)
                        nc.scalar.activation(
                            e_t[:, :w_q], sp[:, :w_q],
                            mybir.ActivationFunctionType.Exp)
                        if j >= 0:  # triangle block only
                            nc.vector.tensor_mul(
                                e_t[:, 0:128], e_t[:, 0:128], mask_sb[:, 384:512])
                        nc.tensor.matmul(
                            att[:, off:512], vblk[h][:, bb * KT + kt, :],
                            e_t[:, :w_q],
                            start=(kt == 0), stop=(kt == nk - 1),
                        )
                        # denominators: non-diagonal tiles pair-summed on
                        # GpSimd; diagonal (ragged) tiles summed individually.
                        # The ones-matmuls run after the kt loop so the
                        # in-order PE stream never waits on the adds.
                        if j < 0:
                            pair_es.append(e_t)
                            if len(pair_es) == 2:
                                tp = exp_pool.tile([P, 512], BF16, name="tp", bufs=10)
                                nc.gpsimd.tensor_add(tp, pair_es[0], pair_es[1])
                                sum_rhs.append((tp, 0))
                                pair_es = []
                        else:
                            sum_rhs.append((e_t, off))
                    for qd, (tq, off) in enumerate(sum_rhs):
                        nc.tensor.matmul(
                            sm[:, off:512], ones_col, tq[:, :512 - off],
                            start=(qd == 0), stop=(qd == len(sum_rhs) - 1),
                        )
                    # reciprocal straight from PSUM (frees the bank), then
                    # broadcast across partitions via a DRAM bounce (DMA can
                    # replicate a DRAM row with a zero-stride partition dim)
                    r = small.tile([1, 512], F32, name="r")
                    nc.vector.reciprocal_approx_fast(out=r, in_=sm)
                    rd = dram_pool.tile([1, 512], F32, name="rd")
                    nc.sync.dma_start(rd, r)
                    rb = small.tile([P, 512], F32, name="rb")
                    rd_b = bass.AP(tensor=rd.tensor, offset=rd.offset,
                                   ap=[[0, P]] + list(rd.ap[1:]))
                    nc.sync.dma_start(rb, rd_b)
                    att_t.append(att)
                    rb_t.append(rb)
                for h in range(hpc):
                    nc.vector.tensor_tensor(
                        attn_sb[h][:, bb * s + qt * 512: bb * s + (qt + 1) * 512],
                        att_t[h], rb_t[h], mybir.AluOpType.mult,
                    )
                if prev_nt is not None:
                    emit_outproj(prev_nt)
                prev_nt = bb * QT + qt
            emit_outproj(prev_nt, alternate=True)

        qk_pool.release()

    nc.compile()
    return nc


def make_common_inputs(x, b=B, s=S, e=E):
    """Inputs identical on every core: xT, trig tables, causal band mask."""
    T = b * s
    KE = e // P
    xflat = np.ascontiguousarray(x.reshape(T, e).T)        # [E, T] f32
    xT = np.ascontiguousarray(
        xflat.reshape(KE, P, T).transpose(1, 0, 2)).astype(ml_dtypes.bfloat16)

    inv_freq = (1.0 / (10000.0 ** (np.arange(0, D, 2, dtype=np.float32) / D)))
    t = np.arange(s, dtype=np.float32)
    freqs = np.outer(t, inv_freq)                           # [S, 64]
    cos = np.cos(freqs).astype(np.float32)                  # [S, 64]
    sin = np.sin(freqs).astype(np.float32)
    cosT = np.concatenate([cos, cos], axis=1).T             # [128, S]
    sinT = np.concatenate([sin, sin], axis=1).T
    sgn = np.where(np.arange(D) < D // 2, -1.0, 1.0).astype(np.float32)[:, None]
    scale = 1.0 / math.sqrt(D)
    cosq = np.ascontiguousarray(cosT * scale).astype(ml_dtypes.bfloat16)
    sinq = np.ascontiguousarray(sinT * sgn * scale).astype(ml_dtypes.bfloat16)
    cosk = np.ascontiguousarray(cosT).astype(ml_dtypes.bfloat16)
    sink = np.ascontiguousarray(sinT * sgn).astype(ml_dtypes.bfloat16)

    r = np.arange(P)[:, None]
    cc = np.arange(896)[None, :]
    bandmask = (cc >= r + 384).astype(ml_dtypes.bfloat16)

    return {
        "xT": xT, "cosq": cosq, "sinq": sinq, "cosk": cosk, "sink": sink,
        "bandmask": bandmask,
    }


def make_core_inputs(W_qkv, W_out, core, b=B, s=S, e=E, hpc=HPC):
    """Per-core column-sharded W_qkv (as lhsT tiles) and row-sharded W_out."""
    KE = e // P
    heads = [core * hpc + i for i in range(hpc)]
    rows = []
    for base in (0, e, 2 * e):  # q, k, v row blocks of W_qkv
        for h in heads:
            rows.append(W_qkv[base + h * D: base + (h + 1) * D])
    Wc = np.concatenate(rows, axis=0)                       # [3*hpc*128, E]
    WcT = np.ascontiguousarray(Wc.T)                        # [E, 3*hpc*128]
    wqkv = np.ascontiguousarray(
        WcT.reshape(KE, P, 3 * hpc * P).transpose(1, 0, 2)).astype(ml_dtypes.bfloat16)

    wo = np.stack(
        [np.ascontiguousarray(W_out[:, h * D:(h + 1) * D].T) for h in heads],
        axis=1)                                             # [128, hpc, E]
    wo = np.ascontiguousarray(wo).astype(ml_dtypes.bfloat16)
    return {"wqkv": wqkv, "wo": wo}


_NC_CACHE = {}


def get_nc():
    key = (B, S, E, HPC)
    if key not in _NC_CACHE:
        _NC_CACHE[key] = build_nc()
    return _NC_CACHE[key]


def kernel(x, W_qkv, W_out):
    x = np.asarray(x, dtype=np.float32)
    W_qkv = np.asarray(W_qkv, dtype=np.float32)
    W_out = np.asarray(W_out, dtype=np.float32)

    nc = get_nc()
    common = make_common_inputs(x)
    in_maps = [dict(common, **make_core_inputs(W_qkv, W_out, c))
               for c in range(N_CORES)]
    res = run_bass_kernel_spmd(nc, in_maps, list(range(N_CORES)))
    total = res.results[0]["outT"].astype(np.float32)
    for c in range(1, N_CORES):
        total = total + res.results[c]["outT"]
    return np.ascontiguousarray(total.T).reshape(B, S, E).astype(np.float32)
